# revision 12
# baseline (speedup 1.0000x reference)
"""CostVolumeLayer Trainium2 kernel.

Computes the local cost volume: for search_range R=4,
  out[b, di*9+dj, i, j] = sum_c src[b,c,i,j] * tgt_zp[b,c,i-2R+di, j-2R+dj]
(tgt zero-padded outside its bounds; the window is OFF-CENTER, covering
tgt rows i-8..i and cols j-8..j — faithful to the torch reference, whose
window indices index the zero-padded tensor directly and whose negative
indices wrap into the zero pad).

Strategy (8 NeuronCores, SPMD):
  - Shard: core c -> batch b = c//2, row-half r0 = 32*(c%2). Each core gets
    src shard [C=128, 32, 128] and a zero-padded tgt halo shard
    [C=128, 40, 136] (host pre-pads; halo = R rows/cols each side), bf16.
  - Device: for each 8x16 pixel block of the shard, one TensorE matmul
    lhsT = src block [K=C=128, M=128 pixels], rhs = tgt window
    [K=128, N=16x24=384] -> PSUM Gram [128, 384]; one full-width
    PSUM->SBUF fp16 copy per block, alternating DVE/ACT engines.
  - Queues (three parallel FIFO DMA queues, one per issuing engine):
    the critical input prefix (first block-row's data) issues from GpSimd
    (SWDGE), whose queue is live ~5us before the HWDGE engines finish
    their preamble; the remaining input chunks issue from Scalar; output
    DMAs issue from Sync. Chunks on one queue drain in issue order, so
    need-order issue = need-order arrival.
  - Band dump: pixel partition p = mi*16+mj only needs Gram cols
    (mi+di)*24+(mj+dj), so partitions 0-63 keep cols 0..287 and
    partitions 64-127 keep cols 96..383: the two output DMAs per group
    read the 288-wide band via a strided AP (25% fewer dump bytes).
  - Host: zero-FLOP banded-diagonal gather from the Gram blocks into the
    [B, 81, H, W] output (the per-pixel diagonal is a per-partition-skewed
    pattern that engine/DMA access patterns cannot express on-chip).
"""

import numpy as np

R = 4
D = 2 * R + 1          # 9
B, C, H, W = 4, 128, 64, 128
NCORES = 8
HS = H // 2            # 32 rows per core shard
TH = HS + 2 * R        # 40 padded tgt rows per shard
TW = W + 2 * R         # 136 padded tgt cols
BI, BJ = 8, 16         # pixel block: 8 rows x 16 cols = 128 = M
NBI, NBJ = HS // BI, W // BJ   # 4 x 8 = 32 blocks per core
WIN_I, WIN_J = BI + 2 * R, BJ + 2 * R  # 16 x 24 window
NW = WIN_I * WIN_J     # 384 streamed columns per block
NBLK = NBI * NBJ
GRP = 8                # blocks per output DMA group (= one block-row)
NGRP = NBLK // GRP     # 4 groups; 2 banded half DMAs each
BANDW = NW - 4 * WIN_J  # 288
BANDO = 4 * WIN_J       # 96, column offset of the upper-half band

_compiled = None


def _build_bass():
    import concourse.mybir as mybir
    from concourse import bacc
    from concourse.tile import TileContext

    f32 = mybir.dt.float32
    in_dt = mybir.dt.bfloat16
    dump_dt = mybir.dt.float16
    nc = bacc.Bacc()
    # single combined input: [C, HS*W (block-reordered src) ++ TH*TW (padded
    # tgt)] in bf16.
    E = HS * W + TH * TW
    inp = nc.dram_tensor("inp", [C, E], in_dt, kind="ExternalInput")
    gout = nc.dram_tensor("gout", [NGRP, 128, GRP * BANDW], dump_dt,
                          kind="ExternalOutput")
    gout_ap = gout.ap()

    with TileContext(nc) as tc:
        with (
            tc.tile_pool(name="inp", bufs=1) as inp_pool,
            tc.tile_pool(name="g", bufs=NGRP) as gpool,
            tc.tile_pool(name="psum", bufs=7, space="PSUM") as psum_pool,
            tc.tile_pool(name="warmpsum", bufs=1, space="PSUM") as warm_pool,
        ):
            # src arrives block-reordered from the host: [C, blk, 128 pixels]
            # so each block's weights are one contiguous free dim.
            a = inp_pool.tile([C, E], in_dt)

            def s_view():
                return a[:, :HS * W]

            def t_view():
                return a[:, HS * W:].rearrange("c (i j) -> c i j", j=TW)

            def in_chunk(lo, hi):
                return nc.sync.dma_start(out=a[:, lo:hi],
                                         in_=inp.ap()[:, lo:hi])

            # Input on the Scalar HWDGE queue (separate FIFO from the Sync
            # queue carrying output DMAs), in need order: block-row r needs
            # src blocks 8r..8r+7 and tgt rows 8r..8r+15 (fine 8-row tgt
            # chunks so each row unblocks as early as possible; coarser
            # chunks measured slower despite fewer receipt stalls).
            HSW = HS * W
            def src_row(r):
                in_chunk(r * 8 * 128, (r + 1) * 8 * 128)
            def tgt_rows(i):
                in_chunk(HSW + i * 8 * TW, HSW + (i + 1) * 8 * TW)
            src_row(0), tgt_rows(0), tgt_rows(1)
            src_row(1), tgt_rows(2)
            src_row(2), tgt_rows(3)
            src_row(3), tgt_rows(4)

            # PE warm-up: dummy matmuls during the input-DMA wait keep the
            # HAM clock gate busy so it flips to 8/8 before the real
            # matmuls (a >=3.4us PE-idle gap re-throttles it to 1.2GHz,
            # measured to slow matmuls from 162ns to ~480ns).
            warm = inp_pool.tile([128, 128], in_dt)
            nc.vector.memset(warm, 0.0)
            wps = warm_pool.tile([1, 128], f32)
            def warm_mms(n):
                for _ in range(n):
                    nc.tensor.matmul(wps, warm[:, :1], warm,
                                     start=True, stop=True)
            warm_mms(32)

            for grp in range(NGRP):
                stage = gpool.tile([128, GRP * NW], dump_dt)
                for k in range(GRP):
                    blk = grp * GRP + k
                    bi, bj = divmod(blk, NBJ)
                    ps = psum_pool.tile([128, NW], f32)
                    lhsT = s_view()[:, blk * 128:(blk + 1) * 128]
                    rhs = t_view()[:, bi * BI: bi * BI + WIN_I,
                                   bj * BJ: bj * BJ + WIN_J]
                    nc.tensor.matmul(ps, lhsT, rhs, start=True, stop=True)
                    # one full-width copy (cheaper than two banded copies:
                    # engine cost is fixed overhead + free-elems), engines
                    # alternated so copies of adjacent blocks run in parallel
                    dst = stage[:, k * NW:(k + 1) * NW]
                    if k % 2 == 0:
                        nc.vector.tensor_copy(dst, ps)
                    else:
                        nc.scalar.copy(dst, ps)
                # banded output DMAs: lower partition half keeps cols
                # 0..287, upper half cols 96..383 of each block.
                sv = stage.rearrange("p (k w) -> p k w", w=NW)
                gv = gout_ap[grp].rearrange("p (k w) -> p k w", w=BANDW)
                nc.sync.dma_start(out=gv[0:64],
                                  in_=sv[0:64, :, 0:BANDW])
                nc.sync.dma_start(out=gv[64:128],
                                  in_=sv[64:128, :, BANDO:BANDO + BANDW])
    nc.finalize()
    return nc


def _get_compiled():
    global _compiled
    if _compiled is None:
        _compiled = _build_bass()
    return _compiled


def _shard_inputs(src, tgt):
    """Build per-core input maps (host-side shard + zero-pad + bf16)."""
    import ml_dtypes

    bf16 = ml_dtypes.bfloat16
    in_maps = []
    for c in range(NCORES):
        b = c // 2
        r0 = HS * (c % 2)
        # block-reorder: [C, NBI, BI, NBJ, BJ] -> [C, (NBI NBJ), (BI BJ)]
        s = np.ascontiguousarray(
            src[b, :, r0:r0 + HS, :]
            .reshape(C, NBI, BI, NBJ, BJ)
            .transpose(0, 1, 3, 2, 4)
        ).reshape(C, HS * W)
        tp = np.zeros((C, TH, TW), dtype=np.float32)
        # The reference's window for output pixel (i, j) covers tgt rows
        # i-2R..i and cols j-2R..j (off-center, faithful to the torch quirk:
        # window indices index the PADDED tensor directly, so padded index
        # i-R+di = tgt row i-2R+di). Device pairs src local row il with
        # shard-padded row il+di, so shard row q holds tgt row r0+q-2R;
        # shard col x holds tgt col x-2R.
        lo = r0 - 2 * R
        hi = r0 + HS
        clo = max(lo, 0)
        tp[:, clo - lo: clo - lo + (hi - clo), 2 * R: 2 * R + W] = tgt[b, :, clo:hi, :]
        inp = np.concatenate([s, tp.reshape(C, TH * TW)], axis=1)
        in_maps.append({"inp": np.ascontiguousarray(inp.astype(bf16))})
    return in_maps


# host-side gather indices: out[k=(di,dj)] at pixel (mi,mj) of a block sits at
# Gram column n = (mi+di)*WIN_J + (mj+dj); the device band dump stores
# columns shifted by BANDO for partition groups mi >= 4.
_mi = np.arange(BI)[:, None, None, None]
_mj = np.arange(BJ)[None, :, None, None]
_di = np.arange(D)[None, None, :, None]
_dj = np.arange(D)[None, None, None, :]
_NIDX = ((_mi + _di) * WIN_J + (_mj + _dj)
         - BANDO * (_mi >= 4)).reshape(BI, BJ, D * D)  # [8,16,81]


def _unshard_output(results):
    out = np.empty((B, D * D, H, W), dtype=np.float32)
    for c in range(NCORES):
        b = c // 2
        r0 = HS * (c % 2)
        g = (results[c]["gout"]
             .astype(np.float32)
             .reshape(NGRP, 128, GRP, BANDW)
             .transpose(0, 2, 1, 3)
             .reshape(NBI, NBJ, BI, BJ, BANDW))
        # gather: v[bi,bj,mi,mj,k] = g[bi,bj,mi,mj,_NIDX[mi,mj,k]]
        v = np.take_along_axis(g, _NIDX[None, None], axis=-1)
        # -> out[b, k, r0+bi*8+mi, bj*16+mj]
        v = v.transpose(4, 0, 2, 1, 3)  # [81, NBI, BI, NBJ, BJ]
        out[b, :, r0:r0 + HS, :] = v.reshape(D * D, HS, W)
    return out


def kernel(src, tgt):
    from concourse.bass_utils import run_bass_kernel_spmd

    src = np.asarray(src, dtype=np.float32)
    tgt = np.asarray(tgt, dtype=np.float32)
    nc = _get_compiled()
    in_maps = _shard_inputs(src, tgt)
    res = run_bass_kernel_spmd(nc, in_maps, core_ids=list(range(NCORES)))
    return _unshard_output(res.results)


# revision 13
# speedup vs baseline: 1.0561x; 1.0561x over previous
"""CostVolumeLayer Trainium2 kernel.

Computes the local cost volume: for search_range R=4,
  out[b, di*9+dj, i, j] = sum_c src[b,c,i,j] * tgt_zp[b,c,i-2R+di, j-2R+dj]
(tgt zero-padded outside its bounds; the window is OFF-CENTER, covering
tgt rows i-8..i and cols j-8..j — faithful to the torch reference, whose
window indices index the zero-padded tensor directly and whose negative
indices wrap into the zero pad).

Strategy (8 NeuronCores, SPMD):
  - Shard: core c -> batch b = c//2, row-half r0 = 32*(c%2). Each core gets
    src shard [C=128, 32, 128] and a zero-padded tgt halo shard
    [C=128, 40, 136] (host pre-pads; halo = R rows/cols each side), bf16.
  - Device: for each 8x16 pixel block of the shard, one TensorE matmul
    lhsT = src block [K=C=128, M=128 pixels], rhs = tgt window
    [K=128, N=16x24=384] -> PSUM Gram [128, 384]; one full-width
    PSUM->SBUF fp16 copy per block, alternating DVE/ACT engines.
  - Queues (three parallel FIFO DMA queues, one per issuing engine):
    the critical input prefix (first block-row's data) issues from GpSimd
    (SWDGE), whose queue is live ~5us before the HWDGE engines finish
    their preamble; the remaining input chunks issue from Scalar; output
    DMAs issue from Sync. Chunks on one queue drain in issue order, so
    need-order issue = need-order arrival.
  - Band dump: pixel partition p = mi*16+mj only needs Gram cols
    (mi+di)*24+(mj+dj), so partitions 0-63 keep cols 0..287 and
    partitions 64-127 keep cols 96..383: the two output DMAs per group
    read the 288-wide band via a strided AP (25% fewer dump bytes).
  - Host: zero-FLOP banded-diagonal gather from the Gram blocks into the
    [B, 81, H, W] output (the per-pixel diagonal is a per-partition-skewed
    pattern that engine/DMA access patterns cannot express on-chip).
"""

import numpy as np

R = 4
D = 2 * R + 1          # 9
B, C, H, W = 4, 128, 64, 128
NCORES = 8
HS = H // 2            # 32 rows per core shard
TH = HS + 2 * R        # 40 padded tgt rows per shard
TW = W + 2 * R         # 136 padded tgt cols
BI, BJ = 8, 16         # pixel block: 8 rows x 16 cols = 128 = M
NBI, NBJ = HS // BI, W // BJ   # 4 x 8 = 32 blocks per core
WIN_I, WIN_J = BI + 2 * R, BJ + 2 * R  # 16 x 24 window
NW = WIN_I * WIN_J     # 384 streamed columns per block
NBLK = NBI * NBJ
GRP = 8                # blocks per output DMA group (= one block-row)
NGRP = NBLK // GRP     # 4 groups; 2 banded half DMAs each
BANDW = NW - 4 * WIN_J  # 288
BANDO = 4 * WIN_J       # 96, column offset of the upper-half band

_compiled = None


def _build_bass():
    import concourse.mybir as mybir
    from concourse import bacc
    from concourse.tile import TileContext

    f32 = mybir.dt.float32
    in_dt = mybir.dt.bfloat16
    dump_dt = mybir.dt.float16
    nc = bacc.Bacc()
    # single combined input: [C, HS*W (block-reordered src) ++ TH*TW (padded
    # tgt)] in bf16.
    E = HS * W + TH * TW
    inp = nc.dram_tensor("inp", [C, E], in_dt, kind="ExternalInput")
    gout = nc.dram_tensor("gout", [NGRP, 128, GRP * BANDW], dump_dt,
                          kind="ExternalOutput")
    gout_ap = gout.ap()

    with TileContext(nc) as tc:
        with (
            tc.tile_pool(name="inp", bufs=1) as inp_pool,
            tc.tile_pool(name="g", bufs=NGRP) as gpool,
            tc.tile_pool(name="psum", bufs=7, space="PSUM") as psum_pool,
            tc.tile_pool(name="warmpsum", bufs=1, space="PSUM") as warm_pool,
        ):
            # src arrives block-reordered from the host: [C, blk, 128 pixels]
            # so each block's weights are one contiguous free dim.
            a = inp_pool.tile([C, E], in_dt)

            def s_view():
                return a[:, :HS * W]

            def t_view():
                return a[:, HS * W:].rearrange("c (i j) -> c i j", j=TW)

            def in_chunk(lo, hi):
                return nc.sync.dma_start(out=a[:, lo:hi],
                                         in_=inp.ap()[:, lo:hi])

            # Input on the Scalar HWDGE queue (separate FIFO from the Sync
            # queue carrying output DMAs), in need order: block-row r needs
            # src blocks 8r..8r+7 and tgt rows 8r..8r+15 (fine 8-row tgt
            # chunks so each row unblocks as early as possible; coarser
            # chunks measured slower despite fewer receipt stalls).
            HSW = HS * W
            def src_row(r):
                in_chunk(r * 8 * 128, (r + 1) * 8 * 128)
            def tgt_rows(i):
                in_chunk(HSW + i * 8 * TW, HSW + (i + 1) * 8 * TW)
            src_row(0), tgt_rows(0), tgt_rows(1)
            src_row(1), tgt_rows(2)
            src_row(2), tgt_rows(3)
            src_row(3), tgt_rows(4)

            # PE warm-up: dummy matmuls during the input-DMA wait keep the
            # HAM clock gate busy so it flips to 8/8 before the real
            # matmuls (a >=3.4us PE-idle gap re-throttles it to 1.2GHz,
            # measured to slow matmuls from 162ns to ~480ns).
            warm = inp_pool.tile([128, 128], in_dt)
            nc.vector.memset(warm, 0.0)
            wps = warm_pool.tile([1, 128], f32)
            def warm_mms(n):
                for _ in range(n):
                    nc.tensor.matmul(wps, warm[:, :1], warm,
                                     start=True, stop=True)
            warm_mms(12)

            for grp in range(NGRP):
                stage = gpool.tile([128, GRP * NW], dump_dt)
                for k in range(GRP):
                    blk = grp * GRP + k
                    bi, bj = divmod(blk, NBJ)
                    ps = psum_pool.tile([128, NW], f32)
                    lhsT = s_view()[:, blk * 128:(blk + 1) * 128]
                    rhs = t_view()[:, bi * BI: bi * BI + WIN_I,
                                   bj * BJ: bj * BJ + WIN_J]
                    nc.tensor.matmul(ps, lhsT, rhs, start=True, stop=True)
                    # one full-width copy (cheaper than two banded copies:
                    # engine cost is fixed overhead + free-elems), engines
                    # alternated so copies of adjacent blocks run in parallel
                    dst = stage[:, k * NW:(k + 1) * NW]
                    if k % 2 == 0:
                        nc.vector.tensor_copy(dst, ps)
                    else:
                        nc.scalar.copy(dst, ps)
                # banded output DMAs: lower partition half keeps cols
                # 0..287, upper half cols 96..383 of each block.
                sv = stage.rearrange("p (k w) -> p k w", w=NW)
                gv = gout_ap[grp].rearrange("p (k w) -> p k w", w=BANDW)
                nc.sync.dma_start(out=gv[0:64],
                                  in_=sv[0:64, :, 0:BANDW])
                nc.sync.dma_start(out=gv[64:128],
                                  in_=sv[64:128, :, BANDO:BANDO + BANDW])
    nc.finalize()
    return nc


def _get_compiled():
    global _compiled
    if _compiled is None:
        _compiled = _build_bass()
    return _compiled


def _shard_inputs(src, tgt):
    """Build per-core input maps (host-side shard + zero-pad + bf16)."""
    import ml_dtypes

    bf16 = ml_dtypes.bfloat16
    in_maps = []
    for c in range(NCORES):
        b = c // 2
        r0 = HS * (c % 2)
        # block-reorder: [C, NBI, BI, NBJ, BJ] -> [C, (NBI NBJ), (BI BJ)]
        s = np.ascontiguousarray(
            src[b, :, r0:r0 + HS, :]
            .reshape(C, NBI, BI, NBJ, BJ)
            .transpose(0, 1, 3, 2, 4)
        ).reshape(C, HS * W)
        tp = np.zeros((C, TH, TW), dtype=np.float32)
        # The reference's window for output pixel (i, j) covers tgt rows
        # i-2R..i and cols j-2R..j (off-center, faithful to the torch quirk:
        # window indices index the PADDED tensor directly, so padded index
        # i-R+di = tgt row i-2R+di). Device pairs src local row il with
        # shard-padded row il+di, so shard row q holds tgt row r0+q-2R;
        # shard col x holds tgt col x-2R.
        lo = r0 - 2 * R
        hi = r0 + HS
        clo = max(lo, 0)
        tp[:, clo - lo: clo - lo + (hi - clo), 2 * R: 2 * R + W] = tgt[b, :, clo:hi, :]
        inp = np.concatenate([s, tp.reshape(C, TH * TW)], axis=1)
        in_maps.append({"inp": np.ascontiguousarray(inp.astype(bf16))})
    return in_maps


# host-side gather indices: out[k=(di,dj)] at pixel (mi,mj) of a block sits at
# Gram column n = (mi+di)*WIN_J + (mj+dj); the device band dump stores
# columns shifted by BANDO for partition groups mi >= 4.
_mi = np.arange(BI)[:, None, None, None]
_mj = np.arange(BJ)[None, :, None, None]
_di = np.arange(D)[None, None, :, None]
_dj = np.arange(D)[None, None, None, :]
_NIDX = ((_mi + _di) * WIN_J + (_mj + _dj)
         - BANDO * (_mi >= 4)).reshape(BI, BJ, D * D)  # [8,16,81]


def _unshard_output(results):
    out = np.empty((B, D * D, H, W), dtype=np.float32)
    for c in range(NCORES):
        b = c // 2
        r0 = HS * (c % 2)
        g = (results[c]["gout"]
             .astype(np.float32)
             .reshape(NGRP, 128, GRP, BANDW)
             .transpose(0, 2, 1, 3)
             .reshape(NBI, NBJ, BI, BJ, BANDW))
        # gather: v[bi,bj,mi,mj,k] = g[bi,bj,mi,mj,_NIDX[mi,mj,k]]
        v = np.take_along_axis(g, _NIDX[None, None], axis=-1)
        # -> out[b, k, r0+bi*8+mi, bj*16+mj]
        v = v.transpose(4, 0, 2, 1, 3)  # [81, NBI, BI, NBJ, BJ]
        out[b, :, r0:r0 + HS, :] = v.reshape(D * D, HS, W)
    return out


def kernel(src, tgt):
    from concourse.bass_utils import run_bass_kernel_spmd

    src = np.asarray(src, dtype=np.float32)
    tgt = np.asarray(tgt, dtype=np.float32)
    nc = _get_compiled()
    in_maps = _shard_inputs(src, tgt)
    res = run_bass_kernel_spmd(nc, in_maps, core_ids=list(range(NCORES)))
    return _unshard_output(res.results)
